# revision 4
# baseline (speedup 1.0000x reference)
"""Mistral GQA attention on 8 Trainium2 NeuronCores.

Sharding: core c -> batch b = c//4, head-group g = c%4.
Each core computes 8 query heads (g*8..g*8+8) and their 2 KV heads
(g*2, g*2+1) for its batch over the full sequence, plus the partial
output projection for its head rows of Wo. Host sums the 4 partial
outputs per batch.

All matmul operands are bf16 (err ~1e-3, well under the 2e-2 gate),
which halves DMA traffic vs f32r at the same PE rate (1 cyc/row for
N>=256 either way). wv and the current xtq half are SBUF-resident to
avoid the baseline's reloads. The softmax denominator is accumulated
on DVE (acc += expS per k-tile) with a single ones-matmul per
(head, q-block) instead of one per k-tile, cutting ~60us of PE time.

Device dataflow (per core):
  A-k: K^T[d,s] = Wk_c^T x^T  (+RoPE)      A-v: V[s,d] = x^T^T Wv_c
  A-q: Q^T[d,s] = Wq_c^T x^T  (+RoPE)      (per S-half)
  B:   S^T[k,q] = K^T^T Q^T; expS = exp(scale*S^T) * causal_mask
       ctxu^T += V^T expS ; acc += expS (DVE); den = 1^T acc
       ctx^T = ctxu * recip(den)
  C:   out^T[e,s] += Wo_c[hd,e]^T ctx^T[hd,s]   -> DRAM (partial, T)
"""
import numpy as np
import ml_dtypes

B, S, E = 2, 2048, 4096
H, KVH, D = 32, 8, 128
GROUPS = H // KVH
N_CORES, TP = 8, 4
HQ = H // TP          # 8 q heads per core
NKV = KVH // TP       # 2 kv heads per core
MAX_POS = 4096
ROPE_THETA = 10000.0
SCALE = float(1.0 / np.sqrt(np.float32(D)))
BF = ml_dtypes.bfloat16

_compiled = None
_last_in_maps = None


# ---------------------------------------------------------------- device ----
def _build_program():
    import concourse.bass as bass  # noqa: F401
    import concourse.mybir as mybir
    from concourse import bacc
    from concourse.tile import TileContext

    F32R = mybir.dt.float32r
    F32 = mybir.dt.float32
    BF16 = mybir.dt.bfloat16
    AF = mybir.ActivationFunctionType

    nc = bacc.Bacc("TRN2", target_bir_lowering=False, debug=False)
    xtq = nc.dram_tensor("xtq", [E, S], BF16, kind="ExternalInput").ap()
    xtk = nc.dram_tensor("xtk", [E, S], BF16, kind="ExternalInput").ap()
    xtv = nc.dram_tensor("xtv", [E, S], BF16, kind="ExternalInput").ap()
    wq = nc.dram_tensor("wq", [E, HQ * D], BF16, kind="ExternalInput").ap()
    wk = nc.dram_tensor("wk", [E, NKV * D], BF16, kind="ExternalInput").ap()
    wv = nc.dram_tensor("wv", [E, NKV * D], BF16, kind="ExternalInput").ap()
    wo = nc.dram_tensor("wo", [32, 128, HQ * D], BF16, kind="ExternalInput").ap()
    cost = nc.dram_tensor("cost", [D, S], BF16, kind="ExternalInput").ap()
    ssin = nc.dram_tensor("ssin", [D, S], BF16, kind="ExternalInput").ap()
    masks = nc.dram_tensor("masks", [D, 512], BF16, kind="ExternalInput").ap()
    outT = nc.dram_tensor("outT", [E, S], BF16, kind="ExternalOutput").ap()

    with TileContext(nc) as tc:
        import contextlib
        with contextlib.ExitStack() as ctx:
            pers = ctx.enter_context(tc.tile_pool(name="pers", bufs=1))
            xs = ctx.enter_context(tc.tile_pool(name="xs", bufs=3))
            ws = ctx.enter_context(tc.tile_pool(name="ws", bufs=3))
            wop = ctx.enter_context(tc.tile_pool(name="wop", bufs=4))
            expp = ctx.enter_context(tc.tile_pool(name="expp", bufs=8))
            ropet = ctx.enter_context(tc.tile_pool(name="ropet", bufs=2))
            accp = ctx.enter_context(tc.tile_pool(name="accp", bufs=3))
            recp = ctx.enter_context(tc.tile_pool(name="recp", bufs=2))
            ostp = ctx.enter_context(tc.tile_pool(name="ostp", bufs=3))
            psp = ctx.enter_context(tc.tile_pool(name="psp", bufs=8, space="PSUM"))

            # ---- persistent tiles
            kt_sb = pers.tile([128, NKV * S], BF16, tag="kt")      # K^T
            v_sb = pers.tile([128, 16 * NKV * D], BF16, tag="v")   # V s-tiles
            wvp = pers.tile([128, 32 * NKV * D], BF16, tag="wvp")  # resident Wv
            xqh = pers.tile([128, 32 * 1024], BF16, tag="xqh")     # x^T q half
            tcos = pers.tile([128, S], BF16, tag="cos")
            tsin = pers.tile([128, S], BF16, tag="sin")
            tmsk = pers.tile([128, 512], BF16, tag="msk")
            ones_f = pers.tile([128, 128], F32, tag="onesf")
            ones_b = pers.tile([128, 128], BF16, tag="onesb")
            qth = pers.tile([128, HQ * 1024], BF16, tag="qth")
            ctxh = pers.tile([128, HQ * 1024], BF16, tag="ctxh")

            def rope_evict(ps, coff, w, dst):
                """dst(bf16) = ps*cos + rot_half(ps)*sin, cols [coff, coff+w).

                ACT copies PSUM->SBUF first so the PSUM bank frees after
                ~0.4us instead of after the whole DVE rope chain."""
                stage = ropet.tile([128, 512], BF16, tag="stage", bufs=5)
                nc.scalar.activation(stage[:, :w], ps[:], AF.Copy)
                t2 = ropet.tile([128, 512], BF16, tag="t2")
                tc_ = ropet.tile([128, 512], BF16, tag="tc")
                nc.vector.tensor_mul(t2[0:64, :w], stage[64:128, :w],
                                     tsin[64:128, coff:coff + w])
                nc.vector.tensor_mul(t2[64:128, :w], stage[0:64, :w],
                                     tsin[0:64, coff:coff + w])
                nc.vector.tensor_mul(tc_[:, :w], stage[:, :w],
                                     tcos[:, coff:coff + w])
                nc.vector.tensor_add(dst, tc_[:, :w], t2[:, :w])

            # ---- phase A-k: K^T (full S), one PSUM generation (8 banks)
            ps_k = [psp.tile([128, 512], F32, tag="acc", name=f"ps_k{i}")
                    for i in range(8)]
            for et in range(32):
                if et == 1:
                    # off the critical path: needed only from evicts on
                    nc.sync.dma_start(tcos[:], cost[:])
                    nc.sync.dma_start(tsin[:], ssin[:])
                    nc.sync.dma_start(tmsk[:], masks[:])
                    nc.gpsimd.memset(ones_f[:], 1.0)
                    nc.vector.tensor_copy(ones_b[:], ones_f[:])
                wk_t = ws.tile([128, NKV * D], BF16, tag="wk")
                nc.sync.dma_start(wk_t[:], wk[et * 128:(et + 1) * 128, :])
                xk_t = xs.tile([128, S], BF16, tag="xk", bufs=2)
                nc.sync.dma_start(xk_t[:], xtk[et * 128:(et + 1) * 128, :])
                for sc in range(4):
                    for db in range(NKV):
                        nc.tensor.matmul(
                            ps_k[db * 4 + sc][:], wk_t[:, db * 128:(db + 1) * 128],
                            xk_t[:, sc * 512:(sc + 1) * 512],
                            start=(et == 0), stop=(et == 31))
            for db in range(NKV):
                for sc in range(4):
                    rope_evict(ps_k[db * 4 + sc][:], sc * 512, 512,
                               kt_sb[:, db * S + sc * 512: db * S + (sc + 1) * 512])

            def load_wvp():
                for et in range(32):
                    nc.sync.dma_start(
                        wvp[:, et * NKV * D:(et + 1) * NKV * D],
                        wv[et * 128:(et + 1) * 128, :])

            def aq_phase(half):
                """Q^T for this half: resident x, 2 gens x (4 heads), 8 banks."""
                for et in range(32):
                    nc.sync.dma_start(
                        xqh[:, et * 1024:(et + 1) * 1024],
                        xtq[et * 128:(et + 1) * 128,
                            half * 1024:(half + 1) * 1024])
                for gen in range(2):
                    hh0 = gen * 4
                    ps_q = [psp.tile([128, 512], F32, tag="acc",
                                     name=f"ps_q{i}") for i in range(8)]
                    for et in range(32):
                        wq_t = ws.tile([128, 512], BF16, tag="wqp")
                        nc.sync.dma_start(
                            wq_t[:], wq[et * 128:(et + 1) * 128,
                                        hh0 * 128: hh0 * 128 + 512])
                        for hq in range(4):
                            for qp in range(2):
                                nc.tensor.matmul(
                                    ps_q[hq * 2 + qp][:],
                                    wq_t[:, hq * 128:(hq + 1) * 128],
                                    xqh[:, et * 1024 + qp * 512:
                                        et * 1024 + (qp + 1) * 512],
                                    start=(et == 0), stop=(et == 31))
                    for hq in range(4):
                        for qp in range(2):
                            coff = half * 1024 + qp * 512
                            dcol = (hh0 + hq) * 1024 + qp * 512
                            rope_evict(ps_q[hq * 2 + qp][:], coff, 512,
                                       qth[:, dcol:dcol + 512])

            def av_stage(stg):
                """V natural [s,d] for s-tiles [stg*4, stg*4+4): 4 banks."""
                ps_v = [psp.tile([128, 256], F32, tag="acc",
                                 name=f"ps_v{stg}_{i}") for i in range(4)]
                for et in range(32):
                    xv_t = xs.tile([128, 512], BF16, tag="xv")
                    nc.sync.dma_start(
                        xv_t[:], xtv[et * 128:(et + 1) * 128,
                                     stg * 512:(stg + 1) * 512])
                    for sti in range(4):
                        nc.tensor.matmul(
                            ps_v[sti][:], xv_t[:, sti * 128:(sti + 1) * 128],
                            wvp[:, et * NKV * D:(et + 1) * NKV * D],
                            start=(et == 0), stop=(et == 31))
                for sti in range(4):
                    st = stg * 4 + sti
                    nc.scalar.activation(v_sb[:, st * 256:(st + 1) * 256],
                                         ps_v[sti][:], AF.Copy)

            def b_block(qt):
                """Attention for one global 512-query block (all heads)."""
                half, qtl = qt // 2, qt % 2
                for hh in range(HQ):
                    kv = hh // GROUPS
                    nkt = 4 * (qt + 1)
                    ps_cu = psp.tile([128, 512], F32, tag="acc")
                    acc = accp.tile([128, 512], BF16, tag="dacc")
                    for kt in range(nkt):
                        j = kt - 4 * qt
                        c0 = max(j, 0) * 128   # masked-zero q-columns skipped
                        w = 512 - c0
                        ps_s = psp.tile([128, 512], F32, tag="acc")
                        nc.tensor.matmul(
                            ps_s[:, c0:512],
                            kt_sb[:, kv * S + kt * 128: kv * S + (kt + 1) * 128],
                            qth[:, hh * 1024 + qtl * 512 + c0:
                                hh * 1024 + (qtl + 1) * 512],
                            start=True, stop=True)
                        te = expp.tile([128, 512], BF16, tag="expS")
                        nc.scalar.activation(te[:, c0:512], ps_s[:, c0:512],
                                             AF.Exp, scale=SCALE)
                        if j >= 0:
                            nc.vector.tensor_mul(
                                te[:, c0:512], te[:, c0:512], tmsk[:, 0:w])
                        nc.tensor.matmul(
                            ps_cu[:, c0:512],
                            v_sb[:, kt * 256 + kv * 128: kt * 256 + (kv + 1) * 128],
                            te[:, c0:512], start=(kt == 0), stop=(kt == nkt - 1))
                        if kt == 0:
                            nc.vector.tensor_copy(acc[:, c0:512], te[:, c0:512])
                        else:
                            nc.vector.tensor_add(acc[:, c0:512], acc[:, c0:512],
                                                 te[:, c0:512])
                    ps_dn = psp.tile([128, 512], F32, tag="acc")
                    nc.tensor.matmul(ps_dn[:], ones_b[:], acc[:],
                                     start=True, stop=True)
                    rec = recp.tile([128, 512], F32, tag="rec")
                    nc.vector.reciprocal(rec[:], ps_dn[:])
                    dcol = hh * 1024 + qtl * 512
                    nc.vector.tensor_mul(ctxh[:, dcol:dcol + 512],
                                         ps_cu[:], rec[:])

            def c_phase(half):
                """out^T partial for this half's s columns."""
                for e32 in range(32):
                    wo_t = wop.tile([128, HQ * 128], BF16, tag="wo")
                    nc.sync.dma_start(wo_t[:], wo[e32])
                    ps_o = [psp.tile([128, 512], F32, tag="acc",
                                     name=f"ps_o{i}") for i in range(2)]
                    for hd in range(HQ):
                        for sl in range(2):
                            nc.tensor.matmul(
                                ps_o[sl][:], wo_t[:, hd * 128:(hd + 1) * 128],
                                ctxh[:, hd * 1024 + sl * 512:
                                     hd * 1024 + (sl + 1) * 512],
                                start=(hd == 0), stop=(hd == HQ - 1))
                    ost = ostp.tile([128, 1024], BF16, tag="ost")
                    for sl in range(2):
                        nc.scalar.activation(ost[:, sl * 512:(sl + 1) * 512],
                                             ps_o[sl][:], AF.Copy)
                    sg = half * 1024
                    nc.sync.dma_start(
                        outT[e32 * 128:(e32 + 1) * 128, sg:sg + 1024], ost[:])

            # phase schedule: A-v stages (4 banks) interleave with the
            # PE-dense early attention blocks of half 0.
            aq_phase(0)
            load_wvp()
            av_stage(0)
            av_stage(1)
            b_block(0)
            av_stage(2)
            b_block(1)
            av_stage(3)
            aq_phase(1)
            c_phase(0)
            b_block(2)
            b_block(3)
            c_phase(1)

    nc.compile()
    return nc


def _get_program():
    global _compiled
    if _compiled is None:
        _compiled = _build_program()
    return _compiled


# ------------------------------------------------------------------ host ----
def _rope_tables_np():
    """Replicate reference._rope_tables in float32 numpy."""
    j = np.arange(0, D, 2, dtype=np.float32)
    inv_freq = (np.float32(1.0) / (np.float32(ROPE_THETA) ** (j / np.float32(D)))
                ).astype(np.float32)
    t = np.arange(MAX_POS, dtype=np.float32)
    freqs = (t[:, None] * inv_freq[None, :]).astype(np.float32)  # [max_pos, D/2]
    emb = np.concatenate([freqs, freqs], axis=-1)                # [max_pos, D]
    return np.cos(emb).astype(np.float32), np.sin(emb).astype(np.float32)


def _numpy_fallback(query, key, value, position_ids, src_mask, Wq, Wk, Wv, Wo):
    cos_t, sin_t = _rope_tables_np()
    pos = np.asarray(position_ids).astype(np.int64)
    cos = cos_t[pos][:, None]
    sin = sin_t[pos][:, None]
    nb, q_len, _ = query.shape
    q = (query @ Wq).reshape(nb, q_len, H, D).transpose(0, 2, 1, 3)
    k = (key @ Wk).reshape(nb, q_len, KVH, D).transpose(0, 2, 1, 3)
    v = (value @ Wv).reshape(nb, q_len, KVH, D).transpose(0, 2, 1, 3)

    def rot(x):
        return np.concatenate([-x[..., D // 2:], x[..., :D // 2]], axis=-1)
    q = q * cos + rot(q) * sin
    k = k * cos + rot(k) * sin
    k = np.repeat(k, GROUPS, axis=1)
    v = np.repeat(v, GROUPS, axis=1)
    out = np.zeros((nb, q_len, E), np.float32)
    for b in range(nb):
        for h in range(H):
            s = (q[b, h] @ k[b, h].T) / np.sqrt(np.float32(D))
            s = np.where(src_mask[b] == 0, np.float32(-1e9), s)
            s = s - s.max(-1, keepdims=True)
            e = np.exp(s)
            a = e / e.sum(-1, keepdims=True)
            ctx = a @ v[b, h]
            out[b] += ctx @ Wo[h * D:(h + 1) * D, :]
    return out


def kernel(query, key, value, position_ids, src_mask, Wq, Wk, Wv, Wo):
    query = np.asarray(query, dtype=np.float32)
    key = np.asarray(key, dtype=np.float32)
    value = np.asarray(value, dtype=np.float32)
    Wq = np.asarray(Wq, dtype=np.float32)
    Wk = np.asarray(Wk, dtype=np.float32)
    Wv = np.asarray(Wv, dtype=np.float32)
    Wo = np.asarray(Wo, dtype=np.float32)
    pos = np.asarray(position_ids).astype(np.int64)
    mask = np.asarray(src_mask)

    causal = np.array_equal(
        mask[0], np.tril(np.ones((S, S), mask.dtype)))
    if causal and mask.shape[0] > 1:
        causal = all(np.array_equal(mask[b], mask[0]) for b in range(1, mask.shape[0]))
    if not causal or query.shape != (B, S, E):
        return _numpy_fallback(query, key, value, pos, mask, Wq, Wk, Wv, Wo)

    from concourse.bass_utils import run_bass_kernel_spmd
    nc = _get_program()

    cos_t, sin_t = _rope_tables_np()
    # single diagonal mask triangle: mask[rk, cq] = 1 if cq >= rk
    rk = np.arange(128)[:, None]
    cq = np.arange(512)[None, :]
    mpat = (cq >= rk).astype(BF)

    in_maps = []
    per_batch = {}
    for b in range(B):
        cosT = np.ascontiguousarray(cos_t[pos[b]].T)         # [D, S]
        sinT = np.ascontiguousarray(sin_t[pos[b]].T)         # [D, S]
        # ssin: rows 0:64 = +sin, rows 64:128 = -sin (see rope_evict)
        ssin = np.concatenate([sinT[:64], -sinT[64:]], axis=0).astype(np.float32)
        per_batch[b] = {
            "xtq": np.ascontiguousarray(query[b].T).astype(BF),
            "xtk": np.ascontiguousarray(key[b].T).astype(BF),
            "xtv": np.ascontiguousarray(value[b].T).astype(BF),
            "cost": cosT.astype(BF),
            "ssin": ssin.astype(BF),
        }
    for c in range(N_CORES):
        b, g = c // TP, c % TP
        in_maps.append({
            **per_batch[b],
            "wq": np.ascontiguousarray(
                Wq[:, g * HQ * D:(g + 1) * HQ * D]).astype(BF),
            "wk": np.ascontiguousarray(
                Wk[:, g * NKV * D:(g + 1) * NKV * D]).astype(BF),
            "wv": np.ascontiguousarray(
                Wv[:, g * NKV * D:(g + 1) * NKV * D]).astype(BF),
            "wo": np.ascontiguousarray(
                Wo[g * HQ * D:(g + 1) * HQ * D, :]
                .reshape(HQ, 128, 32, 128).transpose(2, 1, 0, 3)
                .reshape(32, 128, HQ * 128)).astype(BF),
            "masks": mpat,
        })

    global _last_in_maps
    _last_in_maps = in_maps
    res = run_bass_kernel_spmd(nc, in_maps, core_ids=list(range(N_CORES)))
    out = np.empty((B, S, E), np.float32)
    for b in range(B):
        acc = res.results[b * TP]["outT"].astype(np.float32)
        for g in range(1, TP):
            acc += res.results[b * TP + g]["outT"].astype(np.float32)
        out[b] = acc.T
    return out


if __name__ == "__main__":
    print("building program...")
    _get_program()
    print("built")
